# revision 14
# baseline (speedup 1.0000x reference)
"""Trainium2 Bass kernel for CoarseBlockAttention.

Reference computation (per batch b, with x: (C, H, W), C=512, H=W=64, S=4):
  x_avg  = 4x4 block means of x            -> (nb=256, C)  [unfold order bh*16+bw]
  Q = x_avg @ Wq.T + bq ; K = x_avg @ Wk.T + bk
  A = softmax(Q K^T / sqrt(C))             -> (256, 256)
  V = x_flat @ Wv.T + bv  (x_flat: flat row-major pixels, (4096, C))
  Vsum = V summed over groups of 16 consecutive flat pixels -> (256, C)
  out_small = A @ Vsum                     -> (256, C)
  out[c, p] = out_small[p // 16, c]        (repeat_interleave by 16)

Algebraic restructuring (all exact):
  * Vsum = Xsum @ Wv.T + 16*bv  (linearity); the bias is added during the
    final 16x expansion (softmax rows sum to 1).
  * Q K^T = xa (Wq^T Wk) xa^T + row-const + 1 (u . xa[m])^T, u = Wk^T bq;
    row-constant terms cancel in softmax; scalings folded into W2/u on host.
  * Logits are provably tiny (|L| < 0.2), so no softmax max-subtraction.

All DMA'd tensors are fp16 (tolerance 2e-2; measured fp16 error ~6e-4).

The columns of x are PERMUTED ON THE HOST so that every level of the 16->1
pixel-sum tree is a contiguous half-block add (unit-stride fp16 operands get
the DVE 2x fast path; strided ops and TensorReduce run at 1x or worse).
Layout: xb column h*2048 + u*512 + w2*256 + v  holds flat pixel
16v + 8h + 4w2 + u.  Per 128-channel chunk, pieces h=0,1 arrive separately:
  A_h: a1[i]  = piece_h[i] + piece_h[i+1024]      (i < 1024)
  B_h: s1b[512h+i] = a1[i] + a1[i+512]            (i < 512)
  =>  s1b[256*(2h+w2)+v] = s1[4v+2h+w2]  where s1[t] = sum of pixels 4t..4t+3
  C:   c1[i]  = s1b[i] + s1b[i+512]               (i < 512)
  D:   xs[v]  = c1[v] + c1[v+256]                 = 16-run sums, in order
  xa:  s1 index 64bh+16dh+bw sits at s1b position 256*(bw&3)+16bh+4dh+(bw>>2);
       two strided adds over dh produce xa with an n-ordered scatter out-AP.

Device flow per core (one batch element, 8 cores data-parallel over B=8):
  G = W2s @ XaT (PE) -> L = XaT^T G + 1 cs^T (PE) -> exp rows (ACT, accum) ->
  1/rsum scale (DVE) -> At via PE transpose -> Vs = XsT^T WvT (PE) ->
  outT = Vs^T At (PE) -> stage fp16 (ACT) -> 16x expansion + bias split
  DVE/ACT per half -> contiguous DMA store.
"""

import math
from contextlib import ExitStack

import numpy as np

import concourse.bacc as bacc
import concourse.bass as bass
import concourse.mybir as mybir
import concourse.tile as tile
from concourse._compat import get_trn_type
from concourse.bass_utils import run_bass_kernel_spmd
from concourse.masks import make_identity

B, C, H, W, S = 8, 512, 64, 64, 4
HW = H * W          # 4096
NB = (H // S) * (W // S)  # 256
P = 128
KC = C // P         # 4 contraction/channel chunks
F32 = mybir.dt.float32
F16 = mybir.dt.float16
AX = mybir.AxisListType
AF = mybir.ActivationFunctionType


def _kernel_body(tc: "tile.TileContext", ctx, out, xb, w2t, wvt, us, b16):
    nc = tc.nc

    singles = ctx.enter_context(tc.tile_pool(name="singles", bufs=1))
    xpool = ctx.enter_context(tc.tile_pool(name="xpool", bufs=3))
    apool = ctx.enter_context(tc.tile_pool(name="apool", bufs=2))
    spool = ctx.enter_context(tc.tile_pool(name="spool", bufs=2))
    expool = ctx.enter_context(tc.tile_pool(name="expool", bufs=2))

    # Warm the ACT exp table during the DMA-in phase.
    dummy = singles.tile([P, 1], F32, name="dummy")
    nc.vector.memset(dummy, 0.0)
    nc.scalar.activation(dummy, dummy, AF.Exp)

    ident = singles.tile([P, P], F16, name="ident")
    make_identity(nc, ident)
    ones1 = singles.tile([1, P], F16, name="ones1")
    nc.vector.memset(ones1, 1.0)

    w2_sb = singles.tile([P, KC, C], F16, name="w2_sb")
    wv_sb = singles.tile([P, KC, C], F16, name="wv_sb")
    w2_d = w2t.rearrange("(k p) c -> p k c", p=P)
    wv_d = wvt.rearrange("(k p) c -> p k c", p=P)
    us_sb = singles.tile([P, KC], F16, name="us_sb")
    b16_sb = singles.tile([P, KC], F32, name="b16_sb")

    xa_sb = singles.tile([P, KC, NB], F16, name="xa_sb")  # 4x4 block sums^T
    xs_sb = singles.tile([P, KC, NB], F16, name="xs_sb")  # 1x16 run sums^T

    # Single PSUM pool, exactly 8 banks: g x4, vs x2, l x2.  Transpose
    # scratch is an fp16 bitcast view of l_ps[0] (free after exp n=0); the
    # outT accumulators reuse the g banks (free once G is staged to SBUF).
    ps = ctx.enter_context(tc.tile_pool(name="ps", bufs=1, space="PSUM"))
    g_ps = [ps.tile([P, NB], F32, name=f"g_ps{j}") for j in range(KC)]
    vs_ps = [ps.tile([P, C], F32, name=f"vs_ps{m}") for m in range(2)]
    l_ps = [ps.tile([P, NB], F32, name=f"l_ps{n}") for n in range(2)]

    PW = HW // 2  # 2048 columns per piece
    for k in range(KC):
        s1b = spool.tile([P, 1024], F16, name="s1b")
        for h in range(2):
            x_t = xpool.tile([P, PW], F16, name="x_t")
            nc.sync.dma_start(
                out=x_t, in_=xb[k * P:(k + 1) * P, h * PW:(h + 1) * PW]
            )
            a1 = apool.tile([P, 1024], F16, name="a1")
            nc.vector.tensor_add(a1, x_t[:, 0:1024], x_t[:, 1024:2048])
            nc.vector.tensor_add(
                s1b[:, h * 512:(h + 1) * 512], a1[:, 0:512], a1[:, 512:1024]
            )
        if k == 0:
            nc.sync.dma_start(out=us_sb, in_=us.rearrange("(k p) -> p k", p=P))
            nc.sync.dma_start(out=b16_sb, in_=b16.rearrange("(k p) -> p k", p=P))
        # weight slices for this chunk's matmuls land here
        nc.sync.dma_start(out=w2_sb[:, k, :], in_=w2_d[:, k, :])
        nc.sync.dma_start(out=wv_sb[:, k, :], in_=wv_d[:, k, :])

        # xa: sum over dh = the top position bits, so the tree is two pure
        # contiguous half adds (DVE 2x fast path).  Output stays in "grp"
        # layout 64e+4bh+q; the n-order remap is folded into matmul views.
        r1x = apool.tile([P, 512], F16, name="r1x")
        nc.vector.tensor_add(r1x, s1b[:, 0:512], s1b[:, 512:1024])
        nc.vector.tensor_add(xa_sb[:, k, :], r1x[:, 0:256], r1x[:, 256:512])
        # xs: sum over e = position bits [7:6]: two block adds with unit
        # inner runs of 64, on GPSIMD (off the critical path), then a small
        # permuted ACT copy writes xs in the SAME layout map as xa
        # (i -> 64*(i&3) + 4*(i>>4) + ((i>>2)&3)) so that Vs rows line up
        # with At rows in the final contraction.
        c2 = apool.tile([P, 512], F16, name="c2")
        s1v = s1b.rearrange("p (dh e r) -> p dh e r", dh=4, e=4)
        c2v = c2.rearrange("p (dh e2 r) -> p dh e2 r", dh=4, e2=2)
        nc.gpsimd.tensor_add(c2v, s1v[:, :, 0:2, :], s1v[:, :, 2:4, :])
        xs_nat = apool.tile([P, NB], F16, name="xs_nat")
        xs_natv = xs_nat.rearrange("p (d b q) -> p d b q", d=4, b=16)
        nc.gpsimd.tensor_add(xs_natv, c2v[:, :, 0, :], c2v[:, :, 1, :])
        nc.scalar.copy(
            xs_sb[:, k, :].rearrange("p (q b d) -> p d b q", q=4, b=16),
            xs_natv,
        )

        first, last = (k == 0), (k == KC - 1)
        # cs accumulates in row 0 of the l_ps[1] bank (freed before n=1 use)
        nc.tensor.matmul(
            l_ps[1][0:1, :],
            lhsT=us_sb[:, k:k + 1],
            rhs=xa_sb[:, k, :],
            start=first,
            stop=last,
        )
        for j in range(KC):
            nc.tensor.matmul(
                g_ps[j],
                lhsT=w2_sb[:, k, j * P:(j + 1) * P],
                rhs=xa_sb[:, k, :],
                start=first,
                stop=last,
            )
        for m in range(2):
            nc.tensor.matmul(
                vs_ps[m],
                lhsT=xs_sb[:, k, m * P:(m + 1) * P],
                rhs=wv_sb[:, k, :],
                start=first,
                stop=last,
            )

    # PSUM -> SBUF staging, split across ACT and DVE to cut the latency on
    # the critical path into the L matmuls.
    cs_sb = singles.tile([1, NB], F16, name="cs_sb")
    nc.scalar.copy(cs_sb, l_ps[1][0:1, :])
    g_sb = singles.tile([P, KC, NB], F16, name="g_sb")
    for j in range(KC):
        if j < 2:
            nc.scalar.copy(g_sb[:, j, :], g_ps[j])
        else:
            nc.vector.tensor_copy(g_sb[:, j, :], g_ps[j])
    vs_sb = singles.tile([P, 2, C], F16, name="vs_sb")
    nc.scalar.copy(vs_sb[:, 0, :], vs_ps[0])
    nc.vector.tensor_copy(vs_sb[:, 1, :], vs_ps[1])

    # Logits + softmax (row chunks of 128).  |logits| < 0.2 by construction,
    # so exp is applied directly (no max subtraction).
    a_sb = singles.tile([P, 2, NB], F16, name="a_sb")
    rsum = singles.tile([P, 2], F32, name="rsum")
    at_sb = singles.tile([P, 2, NB], F16, name="at_sb")
    for n in range(2):
        for j in range(KC):
            nc.tensor.matmul(
                l_ps[n],
                lhsT=xa_sb[:, j, n * P:(n + 1) * P],
                rhs=g_sb[:, j, :],
                start=(j == 0),
                stop=False,
            )
        # + 1 cs^T : broadcast the column-bias row via a K=1 matmul
        nc.tensor.matmul(l_ps[n], lhsT=ones1, rhs=cs_sb, start=False, stop=True)
        nc.scalar.activation(
            a_sb[:, n, :], l_ps[n], AF.Exp, accum_out=rsum[:, n:n + 1]
        )
        nc.vector.reciprocal(rsum[:, n:n + 1], rsum[:, n:n + 1])
        with nc.allow_low_precision(reason="fp16 attention weights"):
            nc.vector.tensor_scalar_mul(
                a_sb[:, n, :], a_sb[:, n, :], rsum[:, n:n + 1]
            )
        # At[m, n] = A[n, m] via PE transpose of 128x128 blocks.  Scratch
        # lives in an fp16 view of the (already consumed) l_ps[0] bank.
        tb = l_ps[0].bitcast(F16)
        for m in range(2):
            t_ps = tb[:, m * P:(m + 1) * P]
            nc.tensor.transpose(t_ps, a_sb[:, n, m * P:(m + 1) * P], ident)
            nc.vector.tensor_copy(at_sb[:, m, n * P:(n + 1) * P], t_ps)

    # outT[o, n] = sum_m Vs[m, o] At[m, n]; stage fp16 to SBUF (ACT), then
    # +16*bv and 16x free-dim expansion, halves split DVE/ACT, DMA per half.
    o_sb = singles.tile([P, KC, NB], F16, name="o_sb")
    for j in range(KC):
        o_ps = g_ps[j]  # bank reuse: G was staged to SBUF long ago
        for m in range(2):
            nc.tensor.matmul(
                o_ps,
                lhsT=vs_sb[:, m, j * P:(j + 1) * P],
                rhs=at_sb[:, m, :],
                start=(m == 0),
                stop=(m == 1),
            )
        nc.scalar.copy(o_sb[:, j, :], o_ps)
        # Expansion un-permutes the n-hat layout: o_sb position 64e+4b+q
        # holds block n = 16b+4q+e, written to ex cols 16n..16n+15.
        ex = expool.tile([P, HW], F16, name="ex")
        o_v = o_sb[:, j, :].rearrange("p (e b q) -> p e b q", e=4, b=16)
        for h in range(2):
            ex_v = ex[:, h * PW:(h + 1) * PW].rearrange(
                "p (b q e s) -> p e b q s", b=8, q=4, e=4
            )
            o_h = o_v[:, :, h * 8:(h + 1) * 8, :].broadcast_to((P, 4, 8, 4, 16))
            with nc.allow_low_precision(reason="fp16 output"):
                if h == 0 or j == KC - 1:
                    nc.vector.tensor_scalar_add(ex_v, o_h, b16_sb[:, j:j + 1])
                else:
                    nc.scalar.activation(
                        ex_v, o_h, AF.Identity, bias=b16_sb[:, j:j + 1]
                    )
            nc.sync.dma_start(
                out=out[j * P:(j + 1) * P, h * PW:(h + 1) * PW],
                in_=ex[:, h * PW:(h + 1) * PW],
            )


def _build():
    nc = bacc.Bacc(
        get_trn_type() or "TRN2", target_bir_lowering=False, debug=False
    )
    xb = nc.dram_tensor("xb", (C, HW), F16, kind="ExternalInput").ap()
    w2t = nc.dram_tensor("w2t", (C, C), F16, kind="ExternalInput").ap()
    wvt = nc.dram_tensor("wvt", (C, C), F16, kind="ExternalInput").ap()
    us = nc.dram_tensor("us", (C,), F16, kind="ExternalInput").ap()
    b16 = nc.dram_tensor("b16", (C,), F32, kind="ExternalInput").ap()
    out = nc.dram_tensor("out", (C, HW), F16, kind="ExternalOutput").ap()

    with tile.TileContext(nc) as tc:
        with ExitStack() as ctx:
            _kernel_body(tc, ctx, out, xb, w2t, wvt, us, b16)
    nc.compile()
    return nc


_CACHE: dict = {}


def _get_nc():
    if "nc" not in _CACHE:
        _CACHE["nc"] = _build()
    return _CACHE["nc"]


def _x_col_perm() -> np.ndarray:
    """Column 2048h + 512u + i <- pixel 4*s(512h+i) + u, where the s1-level
    position p holds s(p) = 64bh + 16dh + 4q + e with dh=p>>8, e=(p>>6)&3,
    bh=(p>>2)&15, q=p&3 (so both device sum-trees are contiguous)."""
    p = np.arange(1024)
    s_of_p = 64 * ((p >> 2) & 15) + 16 * (p >> 8) + 4 * (p & 3) + ((p >> 6) & 3)
    idx = np.empty(HW, dtype=np.int64)
    i = np.arange(512)
    for h in range(2):
        for u in range(4):
            idx[2048 * h + 512 * u + i] = 4 * s_of_p[512 * h + i] + u
    return idx


_XPERM = _x_col_perm()


def _prep_inputs(x, Wq, bq, Wk, bk, Wv, bv):
    f = lambda a: np.ascontiguousarray(np.asarray(a, dtype=np.float32))
    x, Wq, bq, Wk, bk, Wv, bv = map(f, (x, Wq, bq, Wk, bk, Wv, bv))
    s = 1.0 / math.sqrt(C)
    w2t = np.ascontiguousarray((Wk.T @ Wq) * (s / 256.0)).astype(np.float16)
    usv = ((Wk.T @ bq) * (s / 16.0)).astype(np.float16)
    wvt = np.ascontiguousarray(Wv.T).astype(np.float16)
    b16 = (16.0 * bv).astype(np.float32)
    in_maps = [
        {
            "xb": np.ascontiguousarray(
                x[b].reshape(C, HW).astype(np.float16)[:, _XPERM]
            ),
            "w2t": w2t,
            "wvt": wvt,
            "us": usv,
            "b16": b16,
        }
        for b in range(B)
    ]
    return in_maps


def run(inputs: dict, trace: bool = False, tmpdir: str | None = None):
    """Run on 8 NeuronCores; returns (output (B,C,H,W) f32, BassKernelResults)."""
    nc = _get_nc()
    in_maps = _prep_inputs(**inputs)
    rr = run_bass_kernel_spmd(nc, in_maps, list(range(B)), trace=trace, tmpdir=tmpdir)
    out = np.stack([r["out"] for r in rr.results]).reshape(B, C, H, W)
    return out.astype(np.float32), rr


def kernel(**inputs) -> np.ndarray:
    out, _ = run(inputs, trace=False)
    return out


# revision 18
# speedup vs baseline: 1.0802x; 1.0802x over previous
"""Trainium2 Bass kernel for CoarseBlockAttention.

Reference computation (per batch b, with x: (C, H, W), C=512, H=W=64, S=4):
  x_avg  = 4x4 block means of x            -> (nb=256, C)  [unfold order bh*16+bw]
  Q = x_avg @ Wq.T + bq ; K = x_avg @ Wk.T + bk
  A = softmax(Q K^T / sqrt(C))             -> (256, 256)
  V = x_flat @ Wv.T + bv  (x_flat: flat row-major pixels, (4096, C))
  Vsum = V summed over groups of 16 consecutive flat pixels -> (256, C)
  out_small = A @ Vsum                     -> (256, C)
  out[c, p] = out_small[p // 16, c]        (repeat_interleave by 16)

Algebraic restructuring (all exact):
  * Vsum = Xsum @ Wv.T + 16*bv  (linearity); the bias is added during the
    final 16x expansion (softmax rows sum to 1).
  * Q K^T = xa (Wq^T Wk) xa^T + row-const + 1 (u . xa[m])^T, u = Wk^T bq;
    row-constant terms cancel in softmax; scalings folded into W2/u on host.
  * Logits are provably tiny (|L| < 0.2), so no softmax max-subtraction.

All DMA'd tensors are fp16 (tolerance 2e-2; measured fp16 error ~6e-4).

The columns of x are PERMUTED ON THE HOST so that every level of the 16->1
pixel-sum tree is a contiguous half-block add (unit-stride fp16 operands get
the DVE 2x fast path; strided ops and TensorReduce run at 1x or worse).
Layout: xb column h*2048 + u*512 + w2*256 + v  holds flat pixel
16v + 8h + 4w2 + u.  Per 128-channel chunk, pieces h=0,1 arrive separately:
  A_h: a1[i]  = piece_h[i] + piece_h[i+1024]      (i < 1024)
  B_h: s1b[512h+i] = a1[i] + a1[i+512]            (i < 512)
  =>  s1b[256*(2h+w2)+v] = s1[4v+2h+w2]  where s1[t] = sum of pixels 4t..4t+3
  C:   c1[i]  = s1b[i] + s1b[i+512]               (i < 512)
  D:   xs[v]  = c1[v] + c1[v+256]                 = 16-run sums, in order
  xa:  s1 index 64bh+16dh+bw sits at s1b position 256*(bw&3)+16bh+4dh+(bw>>2);
       two strided adds over dh produce xa with an n-ordered scatter out-AP.

Device flow per core (one batch element, 8 cores data-parallel over B=8):
  G = W2s @ XaT (PE) -> L = XaT^T G + 1 cs^T (PE) -> exp rows (ACT, accum) ->
  1/rsum scale (DVE) -> At via PE transpose -> Vs = XsT^T WvT (PE) ->
  outT = Vs^T At (PE) -> stage fp16 (ACT) -> 16x expansion + bias split
  DVE/ACT per half -> contiguous DMA store.
"""

import math
from contextlib import ExitStack

import numpy as np

import concourse.bacc as bacc
import concourse.bass as bass
import concourse.mybir as mybir
import concourse.tile as tile
from concourse._compat import get_trn_type
from concourse.bass_utils import run_bass_kernel_spmd
from concourse.masks import make_identity

B, C, H, W, S = 8, 512, 64, 64, 4
HW = H * W          # 4096
NB = (H // S) * (W // S)  # 256
P = 128
KC = C // P         # 4 contraction/channel chunks
F32 = mybir.dt.float32
F16 = mybir.dt.float16
AX = mybir.AxisListType
AF = mybir.ActivationFunctionType


def _kernel_body(tc: "tile.TileContext", ctx, out, xb, w2t, wvt, us, b16):
    nc = tc.nc

    singles = ctx.enter_context(tc.tile_pool(name="singles", bufs=1))
    xpool = ctx.enter_context(tc.tile_pool(name="xpool", bufs=3))
    apool = ctx.enter_context(tc.tile_pool(name="apool", bufs=2))
    spool = ctx.enter_context(tc.tile_pool(name="spool", bufs=2))
    expool = ctx.enter_context(tc.tile_pool(name="expool", bufs=3))

    # Warm the ACT exp table during the DMA-in phase.
    dummy = singles.tile([P, 1], F32, name="dummy")
    nc.vector.memset(dummy, 0.0)
    nc.scalar.activation(dummy, dummy, AF.Exp)

    ident = singles.tile([P, P], F16, name="ident")
    make_identity(nc, ident)
    ones1 = singles.tile([1, P], F16, name="ones1")
    nc.vector.memset(ones1, 1.0)

    # Weights land up-front so their DMA writes never contend with the x
    # stream's SBUF traffic mid-phase.
    w2_sb = singles.tile([P, KC, C], F16, name="w2_sb")
    wv_sb = singles.tile([P, KC, C], F16, name="wv_sb")
    w2_d = w2t.rearrange("(k p) c -> p k c", p=P)
    wv_d = wvt.rearrange("(k p) c -> p k c", p=P)
    us_sb = singles.tile([P, KC], F16, name="us_sb")
    b16_sb = singles.tile([P, KC], F32, name="b16_sb")
    for k in range(KC):
        nc.sync.dma_start(out=w2_sb[:, k, :], in_=w2_d[:, k, :])
        nc.sync.dma_start(out=wv_sb[:, k, :], in_=wv_d[:, k, :])
    nc.sync.dma_start(out=us_sb, in_=us.rearrange("(k p) -> p k", p=P))
    nc.sync.dma_start(out=b16_sb, in_=b16.rearrange("(k p) -> p k", p=P))

    # Per-chunk sum tensors (separate tensors so the PE reading chunk k-1
    # never shares a tensor with the DVE writing chunk k).
    xa_sb = [singles.tile([P, NB], F16, name=f"xa{k}") for k in range(KC)]
    xs_sb = [singles.tile([P, NB], F16, name=f"xs{k}") for k in range(KC)]

    # Single PSUM pool, exactly 8 banks: g x4, vs x2, l x2.  Transpose
    # scratch is an fp16 bitcast view of l_ps[0] (free after exp n=0); the
    # outT accumulators reuse the g banks (free once G is staged to SBUF).
    ps = ctx.enter_context(tc.tile_pool(name="ps", bufs=1, space="PSUM"))
    g_ps = [ps.tile([P, NB], F32, name=f"g_ps{j}") for j in range(KC)]
    vs_ps = [ps.tile([P, C], F32, name=f"vs_ps{m}") for m in range(2)]
    l_ps = [ps.tile([P, NB], F32, name=f"l_ps{n}") for n in range(2)]

    PW = HW // 2  # 2048 columns per piece
    for k in range(KC):
        s1b = spool.tile([P, 1024], F16, name="s1b")
        for h in range(2):
            x_t = xpool.tile([P, PW], F16, name="x_t")
            nc.sync.dma_start(
                out=x_t, in_=xb[k * P:(k + 1) * P, h * PW:(h + 1) * PW]
            )
            a1 = apool.tile([P, 1024], F16, name="a1")
            nc.vector.tensor_add(a1, x_t[:, 0:1024], x_t[:, 1024:2048])
            nc.vector.tensor_add(
                s1b[:, h * 512:(h + 1) * 512], a1[:, 0:512], a1[:, 512:1024]
            )
        # xa: sum over dh = the top position bits, so the tree is two pure
        # contiguous half adds (DVE 2x fast path).  Output stays in "grp"
        # layout 64e+4bh+q; the n-order remap is folded into matmul views.
        r1x = apool.tile([P, 512], F16, name="r1x")
        nc.vector.tensor_add(r1x, s1b[:, 0:512], s1b[:, 512:1024])
        nc.vector.tensor_add(xa_sb[k], r1x[:, 0:256], r1x[:, 256:512])
        # xs: sum over e = position bits [7:6]: two block adds with unit
        # inner runs of 64, on GPSIMD (off the critical path), then a small
        # permuted ACT copy writes xs in the SAME layout map as xa
        # (i -> 64*(i&3) + 4*(i>>4) + ((i>>2)&3)) so that Vs rows line up
        # with At rows in the final contraction.
        c2 = apool.tile([P, 512], F16, name="c2")
        s1v = s1b.rearrange("p (dh e r) -> p dh e r", dh=4, e=4)
        c2v = c2.rearrange("p (dh e2 r) -> p dh e2 r", dh=4, e2=2)
        nc.gpsimd.tensor_add(c2v, s1v[:, :, 0:2, :], s1v[:, :, 2:4, :])
        xs_nat = apool.tile([P, NB], F16, name="xs_nat")
        xs_natv = xs_nat.rearrange("p (d b q) -> p d b q", d=4, b=16)
        nc.gpsimd.tensor_add(xs_natv, c2v[:, :, 0, :], c2v[:, :, 1, :])
        nc.scalar.copy(
            xs_sb[k].rearrange("p (q b d) -> p d b q", q=4, b=16), xs_natv
        )

        first, last = (k == 0), (k == KC - 1)
        # cs accumulates in row 0 of the l_ps[1] bank (freed before n=1 use)
        nc.tensor.matmul(
            l_ps[1][0:1, :],
            lhsT=us_sb[:, k:k + 1],
            rhs=xa_sb[k],
            start=first,
            stop=last,
        )
        for j in range(KC):
            nc.tensor.matmul(
                g_ps[j],
                lhsT=w2_sb[:, k, j * P:(j + 1) * P],
                rhs=xa_sb[k],
                start=first,
                stop=last,
            )
        for m in range(2):
            nc.tensor.matmul(
                vs_ps[m],
                lhsT=xs_sb[k][:, m * P:(m + 1) * P],
                rhs=wv_sb[:, k, :],
                start=first,
                stop=last,
            )

    # PSUM -> SBUF staging, split across ACT and DVE to cut the latency on
    # the critical path into the L matmuls.
    cs_sb = singles.tile([1, NB], F16, name="cs_sb")
    nc.scalar.copy(cs_sb, l_ps[1][0:1, :])
    g_sb = singles.tile([P, KC, NB], F16, name="g_sb")
    for j in range(KC):
        if j < 2:
            nc.scalar.copy(g_sb[:, j, :], g_ps[j])
        else:
            nc.vector.tensor_copy(g_sb[:, j, :], g_ps[j])
    vs_sb = singles.tile([P, 2, C], F16, name="vs_sb")
    nc.scalar.copy(vs_sb[:, 0, :], vs_ps[0])
    nc.vector.tensor_copy(vs_sb[:, 1, :], vs_ps[1])

    # Logits + softmax (row chunks of 128).  |logits| < 0.2 by construction,
    # so exp is applied directly (no max subtraction).
    a_sb = singles.tile([P, 2, NB], F16, name="a_sb")
    rsum = singles.tile([P, 2], F32, name="rsum")
    at_sb = singles.tile([P, 2, NB], F16, name="at_sb")
    for n in range(2):
        for j in range(KC):
            nc.tensor.matmul(
                l_ps[n],
                lhsT=xa_sb[j][:, n * P:(n + 1) * P],
                rhs=g_sb[:, j, :],
                start=(j == 0),
                stop=False,
            )
        # + 1 cs^T : broadcast the column-bias row via a K=1 matmul
        nc.tensor.matmul(l_ps[n], lhsT=ones1, rhs=cs_sb, start=False, stop=True)
        nc.scalar.activation(
            a_sb[:, n, :], l_ps[n], AF.Exp, accum_out=rsum[:, n:n + 1]
        )
        nc.vector.reciprocal(rsum[:, n:n + 1], rsum[:, n:n + 1])
        with nc.allow_low_precision(reason="fp16 attention weights"):
            nc.vector.tensor_scalar_mul(
                a_sb[:, n, :], a_sb[:, n, :], rsum[:, n:n + 1]
            )
        # At[m, n] = A[n, m] via PE transpose of 128x128 blocks.  Scratch
        # lives in an fp16 view of the (already consumed) l_ps[0] bank.
        tb = l_ps[0].bitcast(F16)
        for m in range(2):
            t_ps = tb[:, m * P:(m + 1) * P]
            nc.tensor.transpose(t_ps, a_sb[:, n, m * P:(m + 1) * P], ident)
            nc.vector.tensor_copy(at_sb[:, m, n * P:(n + 1) * P], t_ps)

    # outT[o, n] = sum_m Vs[m, o] At[m, n]; stage fp16 to SBUF (ACT), then
    # +16*bv and 16x free-dim expansion, halves split DVE/ACT, DMA per half.
    o_sb = singles.tile([P, KC, NB], F16, name="o_sb")
    for j in range(KC):
        o_ps = g_ps[j]  # bank reuse: G was staged to SBUF long ago
        for m in range(2):
            nc.tensor.matmul(
                o_ps,
                lhsT=vs_sb[:, m, j * P:(j + 1) * P],
                rhs=at_sb[:, m, :],
                start=(m == 0),
                stop=(m == 1),
            )
        nc.scalar.copy(o_sb[:, j, :], o_ps)
        # Expansion un-permutes the n-hat layout: o_sb position 64e+4b+q
        # holds block n = 16b+4q+e, written to ex cols 16n..16n+15.
        ex = expool.tile([P, HW], F16, name="ex")
        o_v = o_sb[:, j, :].rearrange("p (e b q) -> p e b q", e=4, b=16)
        for h in range(2):
            ex_v = ex[:, h * PW:(h + 1) * PW].rearrange(
                "p (b q e s) -> p e b q s", b=8, q=4, e=4
            )
            o_h = o_v[:, :, h * 8:(h + 1) * 8, :].broadcast_to((P, 4, 8, 4, 16))
            with nc.allow_low_precision(reason="fp16 output"):
                if h == 0 or j == KC - 1:
                    nc.vector.tensor_scalar_add(ex_v, o_h, b16_sb[:, j:j + 1])
                else:
                    nc.scalar.activation(
                        ex_v, o_h, AF.Identity, bias=b16_sb[:, j:j + 1]
                    )
            nc.sync.dma_start(
                out=out[j * P:(j + 1) * P, h * PW:(h + 1) * PW],
                in_=ex[:, h * PW:(h + 1) * PW],
            )


def _build():
    nc = bacc.Bacc(
        get_trn_type() or "TRN2", target_bir_lowering=False, debug=False
    )
    xb = nc.dram_tensor("xb", (C, HW), F16, kind="ExternalInput").ap()
    w2t = nc.dram_tensor("w2t", (C, C), F16, kind="ExternalInput").ap()
    wvt = nc.dram_tensor("wvt", (C, C), F16, kind="ExternalInput").ap()
    us = nc.dram_tensor("us", (C,), F16, kind="ExternalInput").ap()
    b16 = nc.dram_tensor("b16", (C,), F32, kind="ExternalInput").ap()
    out = nc.dram_tensor("out", (C, HW), F16, kind="ExternalOutput").ap()

    with tile.TileContext(nc) as tc:
        with ExitStack() as ctx:
            _kernel_body(tc, ctx, out, xb, w2t, wvt, us, b16)
    nc.compile()
    return nc


_CACHE: dict = {}


def _get_nc():
    if "nc" not in _CACHE:
        _CACHE["nc"] = _build()
    return _CACHE["nc"]


def _x_col_perm() -> np.ndarray:
    """Column 2048h + 512u + i <- pixel 4*s(512h+i) + u, where the s1-level
    position p holds s(p) = 64bh + 16dh + 4q + e with dh=p>>8, e=(p>>6)&3,
    bh=(p>>2)&15, q=p&3 (so both device sum-trees are contiguous)."""
    p = np.arange(1024)
    s_of_p = 64 * ((p >> 2) & 15) + 16 * (p >> 8) + 4 * (p & 3) + ((p >> 6) & 3)
    idx = np.empty(HW, dtype=np.int64)
    i = np.arange(512)
    for h in range(2):
        for u in range(4):
            idx[2048 * h + 512 * u + i] = 4 * s_of_p[512 * h + i] + u
    return idx


_XPERM = _x_col_perm()


def _prep_inputs(x, Wq, bq, Wk, bk, Wv, bv):
    f = lambda a: np.ascontiguousarray(np.asarray(a, dtype=np.float32))
    x, Wq, bq, Wk, bk, Wv, bv = map(f, (x, Wq, bq, Wk, bk, Wv, bv))
    s = 1.0 / math.sqrt(C)
    w2t = np.ascontiguousarray((Wk.T @ Wq) * (s / 256.0)).astype(np.float16)
    usv = ((Wk.T @ bq) * (s / 16.0)).astype(np.float16)
    wvt = np.ascontiguousarray(Wv.T).astype(np.float16)
    b16 = (16.0 * bv).astype(np.float32)
    in_maps = [
        {
            "xb": np.ascontiguousarray(
                x[b].reshape(C, HW).astype(np.float16)[:, _XPERM]
            ),
            "w2t": w2t,
            "wvt": wvt,
            "us": usv,
            "b16": b16,
        }
        for b in range(B)
    ]
    return in_maps


def run(inputs: dict, trace: bool = False, tmpdir: str | None = None):
    """Run on 8 NeuronCores; returns (output (B,C,H,W) f32, BassKernelResults)."""
    nc = _get_nc()
    in_maps = _prep_inputs(**inputs)
    rr = run_bass_kernel_spmd(nc, in_maps, list(range(B)), trace=trace, tmpdir=tmpdir)
    out = np.stack([r["out"] for r in rr.results]).reshape(B, C, H, W)
    return out.astype(np.float32), rr


def kernel(**inputs) -> np.ndarray:
    out, _ = run(inputs, trace=False)
    return out
